# revision 37
# baseline (speedup 1.0000x reference)
"""Trainium2 Bass kernel for GQA attention (B=2, S=2048, D=1024, 16 q heads,
4 kv heads, head_dim 64, RoPE, causal).

Sharding: 8 cores = 2 (batch) x 4 (kv-head groups). Each core computes, for
its batch b and kv group g: the 4 query heads of group g + 1 kv head, plus the
partial output projection y_partial = attn_out_g @ wo[:, g_cols].T.  The host
unshard step sums the 4 partials per batch (the canonical all-reduce of
row-parallel TP, done on host since each core's output is already needed
host-side).

Device-side layout choices (all matmuls contract over the partition dim):
  - x is fed transposed (D on partitions) so QKV projections produce Q^T/K^T
    (head_dim on partitions, seq on free dim).
  - RoPE: wq/wk rows are permuted on host so lanes 0-31 are the "real" pair
    lanes and 32-63 the "imag" lanes; RoPE is then 2 full-width multiplies
    against replicated [c;s;c;s] tiles + 4 narrow combines on the DVE.
    (The permutation cancels in Q.K^T.)
  - Scores are computed as S^T (keys on partitions, queries on free):
    lhsT = K^T block, rhs = Q^T block.  Softmax needs no max-subtraction
    (|scores/8| <~ 3), so exp runs directly on the PSUM scores; the
    denominator is produced by an extra ones-row in the V stationary
    (out row 64 of the PV matmul = sum_l P^T[l, q]).
  - S blocks are paired into 2-bank PSUM tiles so each exp ACTIVATE covers
    (128, 1024) — halves ScalarE instruction + semaphore overhead.
  - Causal mask: matmuls are only emitted for the lower-triangle blocks; the
    128x128 diagonal blocks are masked multiplicatively (tri mask) after exp.
  - Normalization (divide by denominator, which lives along the free dim):
    reciprocal_approx_fast + gpsimd partition_broadcast + one DVE multiply.
"""

import sys

sys.path.insert(0, "/opt/trn_rl_repo")

from contextlib import ExitStack

import ml_dtypes
import numpy as np

import concourse.bass as bass
import concourse.mybir as mybir
import concourse.tile as tile
from concourse import bacc
from concourse.masks import make_identity

# ---------------------------------------------------------------- constants
B, S, D = 2, 2048, 1024
HD = 64
HALF = HD // 2
HKV = 4          # kv heads total
NH = 4           # q heads per core (= NREP)
KVD = HKV * HD   # 256
GO = NH * HD     # 256 output features per group
N_CORES = 8

SB = 512         # q superblock (matmul free dim)
NQS = S // SB    # 4 q superblocks
NLB = S // 128   # 16 key blocks of 128
KCH = D // 128   # 8 contraction chunks for projections

F32 = mybir.dt.float32
BF16 = mybir.dt.bfloat16
SCALE = 1.0 / 8.0  # 1/sqrt(64)
EXP = mybir.ActivationFunctionType.Exp

_CACHE = {}
DEBUG_DUMPS = False  # set True (before get_nc) to add intermediate outputs


# ---------------------------------------------------------------- builder
def _build():
    nc = bacc.Bacc("TRN2", target_bir_lowering=False, debug=False,
                   enable_asserts=False, num_devices=N_CORES)

    xt_d = nc.dram_tensor("xt", [D, S], BF16, kind="ExternalInput").ap()
    wqt_d = nc.dram_tensor("wqt", [D, GO], BF16, kind="ExternalInput").ap()
    wkvt_d = nc.dram_tensor("wkvt", [D, 2 * HD], BF16,
                            kind="ExternalInput").ap()
    wot_d = nc.dram_tensor("wot", [GO, D], BF16, kind="ExternalInput").ap()
    cost_d = nc.dram_tensor("cost", [HALF, S], F32, kind="ExternalInput").ap()
    sint_d = nc.dram_tensor("sint", [HALF, S], F32, kind="ExternalInput").ap()
    tri_d = nc.dram_tensor("tri", [128, 128], BF16, kind="ExternalInput").ap()
    out_d = nc.dram_tensor("out", [S, D], F32, kind="ExternalOutput").ap()
    dbg = {}
    if DEBUG_DUMPS:
        dbg["qT"] = nc.dram_tensor("dbg_qT", [128, 2, S], BF16,
                                   kind="ExternalOutput").ap()
        dbg["kT"] = nc.dram_tensor("dbg_kT", [128, S], BF16,
                                   kind="ExternalOutput").ap()
        dbg["v"] = nc.dram_tensor("dbg_v", [128, NLB, HD + 1], BF16,
                                  kind="ExternalOutput").ap()
        dbg["att"] = nc.dram_tensor("dbg_att", [128, 2, S], BF16,
                                    kind="ExternalOutput").ap()

    with ExitStack() as ctx:
        tc = ctx.enter_context(tile.TileContext(nc))
        _emit(nc, tc, ctx, xt_d, wqt_d, wkvt_d, wot_d, cost_d, sint_d,
              tri_d, out_d, dbg)

    nc.compile()
    return nc


def _emit(nc, tc, ctx, xt_d, wqt_d, wkvt_d, wot_d, cost_d, sint_d,
          tri_d, out_d, dbg={}):
    perm = ctx.enter_context(tc.tile_pool(name="perm", bufs=1))
    pexp = ctx.enter_context(tc.tile_pool(name="pexp", bufs=6))
    ptmp = ctx.enter_context(tc.tile_pool(name="ptmp", bufs=5))
    pout = ctx.enter_context(tc.tile_pool(name="pout", bufs=6))
    pp_mm = ctx.enter_context(tc.tile_pool(name="ppmm", bufs=2, space="PSUM"))

    # ---------------- persistent SBUF tensors
    xt_sb = perm.tile([128, KCH, S], BF16, tag="xt")
    wqt_sb = perm.tile([128, KCH, GO], BF16, tag="wqt")
    wkvt_sb = perm.tile([128, KCH, 2 * HD], BF16, tag="wkvt")
    wot_sb = perm.tile([128, 2, D], BF16, tag="wot")
    cos4_sb = perm.tile([128, S], F32, tag="cos4")       # cos replicated 4x
    sin4_sb = perm.tile([128, S], F32, tag="sin4")       # sin replicated 4x
    tri_sb = perm.tile([128, 128], BF16, tag="tri")
    ident = perm.tile([64, 64], BF16, tag="ident")
    qT_sb = perm.tile([128, 2, S], BF16, tag="qT")       # [hd|hd, mi, s]
    kT_sb = perm.tile([128, S], BF16, tag="kT")          # rows 64-127 = dup
    v_sb = perm.tile([128, NLB, HD + 1], BF16, tag="v")  # [l, lb, hd|1]
    att_sb = perm.tile([128, 2, S], BF16, tag="att")     # [o%128, o//128, s]
    ones_sb = perm.tile([128, HD], F32, tag="ones")      # K=1 bcast stationary

    # ---------------- input DMAs (critical-path first: wq + x chunk 0)
    xt_r = xt_d.rearrange("(kc p) s -> p kc s", p=128)
    nc.sync.dma_start(wkvt_sb[:],
                      wkvt_d.rearrange("(kc p) m -> p kc m", p=128))
    nc.sync.dma_start(xt_sb[:, 0:4, 0:SB], xt_r[:, 0:4, 0:SB])
    nc.sync.dma_start(xt_sb[:, 4:8, 0:SB], xt_r[:, 4:8, 0:SB])
    nc.gpsimd.dma_start(wqt_sb[:],
                        wqt_d.rearrange("(kc p) m -> p kc m", p=128))
    nc.gpsimd.dma_start(cos4_sb[0:32, :], cost_d)
    nc.gpsimd.dma_start(sin4_sb[0:32, :], sint_d)
    for q in range(1, 4):  # replicate on-chip (saves 1.5 MB of HBM reads)
        nc.gpsimd.dma_start(cos4_sb[q * 32:(q + 1) * 32, :], cos4_sb[0:32, :])
        nc.gpsimd.dma_start(sin4_sb[q * 32:(q + 1) * 32, :], sin4_sb[0:32, :])
    nc.gpsimd.dma_start(tri_sb[:], tri_d)
    nc.sync.dma_start(xt_sb[:, :, SB:2 * SB], xt_r[:, :, SB:2 * SB])
    nc.gpsimd.dma_start(wot_sb[:], wot_d.rearrange("(oc p) d -> p oc d", p=128))
    for si in range(2, NQS):
        nc.sync.dma_start(xt_sb[:, :, si * SB:(si + 1) * SB],
                          xt_r[:, :, si * SB:(si + 1) * SB])
    make_identity(nc, ident[:])
    nc.vector.memset(ones_sb[:], 1.0)

    # ---------------- helper: RoPE on a psum projection tile
    # ps rows per 64-row head block: [real(32); imag(32)].  m0 = ps*cos in
    # SBUF, m1 = ps*sin in PSUM; each combine then mixes one SBUF operand
    # with one PSUM operand so the cross-partition pairing stays legal
    # (walrus requires all SBUF APs of a DVE op on identical partitions).
    def rope(ps, nrow, cols, dst, pool_m1):
        m0 = ptmp.tile([128, SB], BF16, tag="ropem0", name="m0")[0:nrow]
        m1 = pool_m1.tile([128, SB], F32, tag="mm", name="m1")[0:nrow]
        nc.vector.tensor_mul(m0[:], ps, cos4_sb[0:nrow, cols])
        nc.vector.tensor_mul(m1[:], ps, sin4_sb[0:nrow, cols])
        for b0 in range(0, nrow, 64):
            # out_r = r*c - i*s ; out_i = r*s + i*c
            nc.vector.tensor_sub(dst[b0:b0 + 32], m0[b0:b0 + 32, :],
                                 m1[b0 + 32:b0 + 64, :])
            nc.vector.tensor_add(dst[b0 + 32:b0 + 64], m1[b0:b0 + 32, :],
                                 m0[b0 + 32:b0 + 64, :])

    # ---------------- fused pipeline: projections(s) then attention(qs=s)
    nc.vector.memset(v_sb[:, :, HD:HD + 1], 1.0)  # ones column -> denom

    _pend = {}

    def proj_q_a(si, mi, scratch=None):
        cols = slice(si * SB, (si + 1) * SB)
        ps = pp_mm.tile([128, SB], F32, tag="mm", name="psq")
        _pend[("q", si, mi)] = ps
        for kc in range(KCH // 2):
            nc.tensor.matmul(
                ps[:], wqt_sb[:, kc, mi * 128:(mi + 1) * 128],
                xt_sb[:, kc, cols], start=(kc == 0), stop=False)

    def proj_q_b(si, mi, scratch=None):
        cols = slice(si * SB, (si + 1) * SB)
        ps = _pend.pop(("q", si, mi))
        for kc in range(KCH // 2, KCH):
            nc.tensor.matmul(
                ps[:], wqt_sb[:, kc, mi * 128:(mi + 1) * 128],
                xt_sb[:, kc, cols], start=False, stop=(kc == KCH - 1))
        rope(ps[:], 128, cols, qT_sb[:, mi, cols], scratch or pp_mm)

    def proj_kv_a(si, scratch=None):
        cols = slice(si * SB, (si + 1) * SB)
        ps = pp_mm.tile([128, SB], F32, tag="mm", name="pskv")
        _pend[("kv", si)] = ps
        for kc in range(KCH // 2):
            nc.tensor.matmul(ps[:], wkvt_sb[:, kc, :], xt_sb[:, kc, cols],
                             start=(kc == 0), stop=False)

    def proj_kv_b(si, scratch=None):
        cols = slice(si * SB, (si + 1) * SB)
        ps = _pend[("kv", si)]
        for kc in range(KCH // 2, KCH):
            nc.tensor.matmul(ps[:], wkvt_sb[:, kc, :], xt_sb[:, kc, cols],
                             start=False, stop=(kc == KCH - 1))
        rope(ps[0:64], 64, cols, kT_sb[0:64, cols], scratch or pp_mm)
        # duplicate K^T to partitions 64-127 for the odd-head row-tiled mms
        nc.sync.dma_start(kT_sb[64:128, cols], kT_sb[0:64, cols])

    def proj_v_tail(si, scratch=None):
        ps = _pend.pop(("kv", si))
        vt = ptmp.tile([64, SB], BF16, tag="vtstage")
        nc.vector.tensor_copy(vt[:], ps[64:128])
        for j in range(SB // 128):
            lb = si * (SB // 128) + j
            pt = (scratch or pp_mm).tile([128, 64],
                                         BF16, tag="mm", name="pt")
            nc.tensor.transpose(pt[:], vt[:, j * 128:(j + 1) * 128], ident[:])
            nc.vector.tensor_copy(v_sb[:, lb, 0:HD], pt[:])

    def proj_pieces(si):
        return [(proj_kv_a, (si,)), (proj_kv_b, (si,)),
                (proj_v_tail, (si,)),
                (proj_q_a, (si, 0)), (proj_q_b, (si, 0)),
                (proj_q_a, (si, 1)), (proj_q_b, (si, 1))]

    def proj_chunk(si, scratch=None):
        for fn, args in proj_pieces(si):
            fn(*args, scratch)

    def wo_half(si, dh):
        """half of the output projection for one 128-row q chunk"""
        scols = slice(si * 128, (si + 1) * 128)
        if dh == 0:
            ysbs[si] = pout.tile([128, D], F32, tag="ysb", name="ysb")
        ysb = ysbs[si]
        yp = pp_mm.tile([128, 512], F32, tag="mm", name="yp")
        for oc in range(2):
            nc.tensor.matmul(
                yp[:], att_sb[:, oc, scols],
                wot_sb[:, oc, dh * 512:(dh + 1) * 512],
                start=(oc == 0), stop=(oc == 1))
        nc.vector.tensor_copy(ysb[:, dh * 512:(dh + 1) * 512], yp[:])
        nc.sync.dma_start(out_d[scols, dh * 512:(dh + 1) * 512],
                          ysb[:, dh * 512:(dh + 1) * 512])

    ysbs = {}

    # prologue: one projection chunk; proj(1) interleaves into attention(0)
    with tc.tile_pool(name="ppro", bufs=2, space="PSUM") as ppro:
        proj_chunk(0, ppro)
    pp_sb = ctx.enter_context(tc.tile_pool(name="ppsb", bufs=2, space="PSUM"))
    pp_pv = ctx.enter_context(tc.tile_pool(name="pppv", bufs=2, space="PSUM"))

    def norm_recip(db):
        nc.vector.reciprocal(db[:], db[:])

    def norm_head(qs, h, db, aus):
        # partition-broadcast 1/denom via a K=1 outer product on TensorE
        # (PSUM operands are partition-unconstrained in the multiply)
        qcols = slice(qs * SB, (qs + 1) * SB)
        mi, i = h // 2, h % 2
        b0 = i * 64
        r = 32 * h
        rbp = pp_mm.tile([HD, SB], F32, tag="mm", name="rbp")
        nc.tensor.matmul(rbp[:], ones_sb[r:r + 1, :], db[r:r + 1, :],
                         start=True, stop=True,
                         tile_position=(r, 0) if r == 96 else None)
        nc.vector.tensor_mul(
            att_sb[b0:b0 + 64, mi, qcols], aus[mi][b0:b0 + 64, :], rbp[:])

    prev_norm = None  # (db, aus) of the previous superblock
    fillers = []       # (deadline_qs, fn, args) — popped one per group

    for qs in range(NQS):
        qcols = slice(qs * SB, (qs + 1) * SB)
        nlb = 4 * qs + 4          # key blocks needed (block-causal)
        # PE filler work popped between attention groups: the previous
        # superblock's (deferred) normalization + output projection, and the
        # (qs+2) projection chunk.
        # flush any overdue fillers (projections for THIS superblock)
        while fillers and fillers[0][0] <= qs:
            _, fn, args = fillers.pop(0)
            fn(*args)
        if prev_norm is not None:
            pdb, paus = prev_norm
            fillers.append((NQS, norm_recip, (pdb,)))
            for h in range(NH):
                fillers.append((NQS, norm_head, (qs - 1, h, pdb, paus)))
            for sj in range(4):
                for dh in range(2):
                    fillers.append((NQS, wo_half, ((qs - 1) * 4 + sj, dh)))
        if qs == 0:
            fillers.extend((1, fn, a) for fn, a in proj_pieces(1))
        if qs + 2 < NQS:
            fillers.extend((qs + 2, fn, a) for fn, a in proj_pieces(qs + 2))
        # unnormalized attention rows + denominators for this superblock are
        # staged to SBUF immediately so the PV psum slots recycle fast and
        # normalization runs off the critical path (deferred into qs+1).
        db = ptmp.tile([128, SB], F32, tag="denom", name="db")
        nc.vector.memset(db[:], 1.0)  # rows 32h get denominators; rest stay 1
        aus = []

        for mi in range(2):       # head pair (2mi, 2mi+1) at partitions 0/64
            po0 = pp_pv.tile([HD + 1, SB], F32, tag="pv", name="po0")
            po1 = pp_pv.tile([HD + 1, SB], F32, tag="pv", name="po1")
            pos = (po0, po1)
            for lb in range(nlb):
                j = lb - 4 * qs   # >=0 on the diagonal superblock
                kcols = slice(lb * 128, (lb + 1) * 128)
                sp = pp_sb.tile([128, 2, SB], F32, tag="sbig", name="sp")
                # the two matmuls run concurrently (row groups 0-1 / 2-3)
                nc.tensor.matmul(sp[:, 0, :], kT_sb[0:64, kcols],
                                 qT_sb[0:64, mi, qcols], start=True, stop=True)
                nc.tensor.matmul(sp[:, 1, :], kT_sb[64:128, kcols],
                                 qT_sb[64:128, mi, qcols], start=True, stop=True)
                pe = pexp.tile([128, 2, SB], BF16, tag="pexp", name="pe")
                if j < 0:
                    nc.scalar.activation(pe[:], sp[:], EXP, scale=SCALE)
                else:
                    ecols = slice(j * 128, SB)
                    nc.scalar.activation(pe[:, :, ecols], sp[:, :, ecols],
                                         EXP, scale=SCALE)
                if j >= 0:
                    dcols = slice(j * 128, (j + 1) * 128)
                    nc.vector.tensor_mul(pe[:, 0, dcols], pe[:, 0, dcols],
                                         tri_sb[:])
                    nc.vector.tensor_mul(pe[:, 1, dcols], pe[:, 1, dcols],
                                         tri_sb[:])
                vcols = slice(max(j, 0) * 128, SB)
                for i in range(2):
                    nc.tensor.matmul(pos[i][:, vcols], v_sb[:, lb, :],
                                     pe[:, i, vcols],
                                     start=(lb == 0), stop=(lb == nlb - 1))
                if fillers:
                    _, fn, args = fillers.pop(0)
                    fn(*args)
            au = ptmp.tile([128, SB], F32, tag="aun", name="au")
            aus.append(au)
            for i in range(2):
                b0 = i * 64
                r = 32 * (2 * mi + i)
                nc.vector.tensor_copy(db[r:r + 1, :], pos[i][HD:HD + 1, :])
                nc.vector.tensor_copy(au[b0:b0 + 64, :], pos[i][0:HD, :])
        prev_norm = (db, aus)
    # epilogue: leftovers + last superblock's normalization + output proj
    for _, fn, args in fillers:
        fn(*args)
    pdb, paus = prev_norm
    norm_recip(pdb)
    for h in range(NH):
        norm_head(3, h, pdb, paus)
    for sj in range(4):
        wo_half(3 * 4 + sj, 0)
        wo_half(3 * 4 + sj, 1)
    if dbg:
        nc.sync.dma_start(dbg["qT"], qT_sb[:])
        nc.sync.dma_start(dbg["kT"], kT_sb[:])
        nc.sync.dma_start(dbg["v"], v_sb[:])
        nc.sync.dma_start(dbg["att"], att_sb[:])


# ---------------------------------------------------------------- host side
def _prep_inputs(x, wq, wk, wv, wo, freqs_cos, freqs_sin):
    """Shard + lay out host-side. Returns list of 8 in_maps."""
    bf = ml_dtypes.bfloat16
    # even/odd pair permutation within each head's 64 rows
    perm = np.concatenate([np.arange(0, HD, 2), np.arange(1, HD, 2)])
    cost = np.ascontiguousarray(freqs_cos.T).astype(np.float32)
    sint = np.ascontiguousarray(freqs_sin.T).astype(np.float32)
    tri = np.triu(np.ones((128, 128), np.float32)).astype(bf)  # [l, q]: l <= q

    in_maps = []
    for c in range(N_CORES):
        b, g = divmod(c, 4)
        xt = np.ascontiguousarray(x[b].T).astype(bf)
        wq_g = wq[g * GO:(g + 1) * GO].reshape(NH, HD, D)[:, perm, :].reshape(GO, D)
        wqt = np.ascontiguousarray(wq_g.T).astype(bf)
        wkt = wk[g * HD:(g + 1) * HD][perm].T
        wvt = wv[g * HD:(g + 1) * HD].T
        wkvt = np.ascontiguousarray(np.concatenate([wkt, wvt], 1)).astype(bf)
        wot = np.ascontiguousarray(wo[:, g * GO:(g + 1) * GO].T).astype(bf)
        in_maps.append({
            "xt": xt, "wqt": wqt, "wkvt": wkvt, "wot": wot,
            "cost": cost, "sint": sint, "tri": tri,
        })
    return in_maps


def get_nc():
    if "nc" not in _CACHE:
        _CACHE["nc"] = _build()
    return _CACHE["nc"]


def _ensure_ntff_hook():
    """The image's antenv lacks axon_hooks; inject an equivalent module so
    run_bass_kernel_spmd(trace=True) can capture NTFF profiles via the
    libaxon_pjrt.so C ABI (same shim trn_boot would register)."""
    import sys as _sys
    import types
    if "antenv.axon_hooks" in _sys.modules:
        return
    import contextlib
    import ctypes

    def _make_hook(so_path="/opt/axon/libaxon_pjrt.so"):
        try:
            lib = ctypes.CDLL(so_path)
        except OSError:
            return None
        if not hasattr(lib, "axon_start_nrt_profile"):
            return None
        lib.axon_start_nrt_profile.argtypes = [ctypes.POINTER(ctypes.c_int64),
                                               ctypes.c_size_t]
        lib.axon_start_nrt_profile.restype = ctypes.c_int64
        lib.axon_stop_nrt_profile.argtypes = [ctypes.c_char_p]
        lib.axon_stop_nrt_profile.restype = ctypes.c_int64

        @contextlib.contextmanager
        def _hook(output_dir, device_ids):
            import jax
            jax.devices()
            if device_ids:
                ids = (ctypes.c_int64 * len(device_ids))(*device_ids)
                rc = lib.axon_start_nrt_profile(ids, len(device_ids))
            else:
                rc = lib.axon_start_nrt_profile(None, 0)
            if rc != 0:
                raise RuntimeError(f"axon_start_nrt_profile rc={rc}")
            try:
                yield
            finally:
                n = lib.axon_stop_nrt_profile(str(output_dir).encode())
                print(f"profile: {n} file(s) -> {output_dir}", file=sys.stderr)

        return _hook

    hook = _make_hook()
    mod = types.ModuleType("antenv.axon_hooks")
    mod.get_axon_ntff_profile_hook = lambda: hook
    mod.set_axon_ntff_profile_hook = lambda h: None
    _sys.modules["antenv.axon_hooks"] = mod


def run(inputs, trace=False):
    from concourse.bass_utils import run_bass_kernel_spmd
    if trace:
        _ensure_ntff_hook()
    nc = get_nc()
    in_maps = _prep_inputs(**inputs)
    res = run_bass_kernel_spmd(nc, in_maps, core_ids=list(range(N_CORES)),
                               trace=trace)
    return res


def kernel(**inputs) -> np.ndarray:
    res = run(inputs)
    outs = [r["out"] for r in res.results]
    y = np.stack([outs[4 * b] + outs[4 * b + 1] + outs[4 * b + 2] + outs[4 * b + 3]
                  for b in range(B)])
    return y.astype(np.float32)
